# revision 1
# baseline (speedup 1.0000x reference)
"""Bidirectional LSTM encoder (nn_BiEncode) as a Bass/Tile kernel on 8 trn2 cores.

Sharding: direction-split x batch-split. Cores 0-3 run the LEFT (forward-time)
direction on batch shards 0-3 (512 rows each); cores 4-7 run the RIGHT
direction (time-reversed input, handled host-side) on the same batch shards.
Every core runs the identical SPMD program; direction differences live
entirely in the data it is fed (weights + time-reversed x).

Device layout: everything is kept "transposed" (feature dim on partitions,
batch on the free dim) so the scan needs no on-chip transposes:
  x fed as xT[t, i, b], weights as W^T, h/c as [H, B] tiles, output written
  as yT[t, h, b] and un-transposed on the host.

Per timestep the full gate pre-activation g^T[4H, B] is computed as 12
PSUM-accumulated matmuls per 128-row gate tile (8 k-tiles of x-projection +
4 k-tiles of the recurrent term) -- the input projection is fused into the
scan, so no pre-activation tensor is ever materialized. ACT applies
sigmoid/tanh straight out of PSUM; DVE does the cell update.
"""

import os

import numpy as np

FRAME_LENGTH = 26
HIDDEN = 512
INPUT = 1024
BATCH = 2048

NCORES = 8
NSHARD = 4                 # batch shards per direction group
BC = BATCH // NSHARD       # 512 batch rows per core

P = 128
KI = INPUT // P            # 8  k-tiles for the input projection
KH = HIDDEN // P           # 4  k-tiles for the recurrent matmul
NJ = HIDDEN // P           # 4  hidden chunks
NM = 4 * HIDDEN // P       # 16 gate m-tiles

# "f32r": fp32 storage, PE in float32r (full-rate at N>=256, ~tf32 precision)
# "bf16": bf16 storage+PE (half DMA/SBUF), fp32 PSUM accumulation
MM_MODE = os.environ.get("BASS_LSTM_MM", "f32r")

_CACHE = {}


def _build(T, Bc, mode):
    import concourse.mybir as mybir
    import concourse.tile as tile
    from concourse import bacc

    dt = mybir.dt
    AF = mybir.ActivationFunctionType

    # matmul-operand storage dtype; the BIR verifier requires fp32r matmul
    # inputs to be produced as fp32r, so x/w/h carry it end-to-end
    io_dt = dt.bfloat16 if mode == "bf16" else dt.float32r

    nc = bacc.Bacc("TRN2", target_bir_lowering=False, debug=False,
                   num_devices=NCORES)
    xT = nc.dram_tensor("xT", [T, INPUT, Bc], io_dt, kind="ExternalInput").ap()
    w_ih = nc.dram_tensor("w_ih", [INPUT, 4 * HIDDEN], io_dt,
                          kind="ExternalInput").ap()
    w_hh = nc.dram_tensor("w_hh", [HIDDEN, 4 * HIDDEN], io_dt,
                          kind="ExternalInput").ap()
    bias = nc.dram_tensor("bias", [P, NM], dt.float32, kind="ExternalInput").ap()
    out_dt = io_dt
    yT = nc.dram_tensor("yT", [T, HIDDEN, Bc], out_dt, kind="ExternalOutput").ap()

    with tile.TileContext(nc) as tc:
        with tc.tile_pool(name="wpool", bufs=1) as wp, \
             tc.tile_pool(name="xpool", bufs=2) as xp, \
             tc.tile_pool(name="state", bufs=2) as sp, \
             tc.tile_pool(name="gates", bufs=2) as gp, \
             tc.tile_pool(name="tmp", bufs=2) as tp, \
             tc.tile_pool(name="psum", bufs=2, space="PSUM") as pp:

            # Startup-latency ordering: the t=0 x tile and the j=0 weight
            # chunk are DMA'd first so the PE can start ~35us earlier than
            # if all weights were queued at once (the SDMA rings drain all
            # queued transfers at a fair share, so everything would land
            # together). Host pre-permutes W columns j-major (see
            # _prep_inputs) so each j-chunk is one contiguous DMA.
            w_ih_r = w_ih.rearrange("(k p) n -> p k n", p=P)
            w_hh_r = w_hh.rearrange("(k p) n -> p k n", p=P)

            bias_sb = wp.tile([P, NM], dt.float32, tag="bias")
            nc.sync.dma_start(out=bias_sb, in_=bias[:, :])

            # Startup ramp: t=0 x and w_ih arrive as 256KB (j,k)-slices so
            # the PE can start on the first slice ~13us in and never waits
            # for a 2MB chunk cliff; emission order puts the slices needed
            # first at the head of each DMA lane's FIFO.
            xt0k = []
            w_ih_sb = [[None] * KI for _ in range(NJ)]
            for k in range(KI):
                xk = wp.tile([P, Bc], io_dt, tag=f"x0_{k}")
                nc.sync.dma_start(
                    out=xk, in_=xT[0, k * P:(k + 1) * P, :])
                xt0k.append(xk)
                wt = wp.tile([P, 4 * P], io_dt, tag=f"wih0_{k}")
                nc.sync.dma_start(out=wt, in_=w_ih_r[:, k, 0:4 * P])
                w_ih_sb[0][k] = wt
            # prefetch t=1's x before the remaining weight triggers queue up
            xt1 = None
            if T > 1:
                xt1 = xp.tile([P, KI, Bc], io_dt, tag="x")
                nc.sync.dma_start(
                    out=xt1, in_=xT[1].rearrange("(k p) b -> p k b", p=P))
            # w_hh triggers ride the scalar engine's HWDGE queue in parallel
            # with the sync-engine trigger stream
            w_hh_sb = []
            for j in range(NJ):
                wt = wp.tile([P, KH, 4 * P], io_dt, tag=f"whh{j}")
                nc.scalar.dma_start(
                    out=wt, in_=w_hh_r[:, :, j * 4 * P:(j + 1) * 4 * P])
                w_hh_sb.append(wt)
            for j in range(1, NJ):
                for k in range(KI):
                    wt = wp.tile([P, 4 * P], io_dt, tag=f"wih{j}_{k}")
                    nc.sync.dma_start(
                        out=wt, in_=w_ih_r[:, k, j * 4 * P:(j + 1) * 4 * P])
                    w_ih_sb[j][k] = wt

            # h0 = c0 = 0, so step 0 skips the recurrent matmuls and the
            # f*c term entirely -- no initial state tiles needed (memset
            # can't produce float32r anyway).
            h_cur, c_cur = [], []

            GATE_FUNCS = (AF.Sigmoid, AF.Sigmoid, AF.Tanh, AF.Sigmoid)

            for t in range(T):
                if t == 1:
                    xt = xt1
                elif t > 1:
                    xt = xp.tile([P, KI, Bc], io_dt, tag="x")
                    nc.sync.dma_start(
                        out=xt, in_=xT[t].rearrange("(k p) b -> p k b", p=P))

                h_next, c_next = [], []
                for j in range(NJ):
                    acts = []
                    for gi in range(4):
                        m = gi * NJ + j
                        ps = pp.tile([P, Bc], dt.float32, tag=f"ps{gi}")
                        for k in range(KI):
                            nc.tensor.matmul(
                                ps, lhsT=w_ih_sb[j][k][:, gi * P:(gi + 1) * P],
                                rhs=(xt0k[k] if t == 0 else xt[:, k, :]),
                                start=(k == 0),
                                stop=(t == 0 and k == KI - 1))
                        if t > 0:
                            for k in range(KH):
                                nc.tensor.matmul(
                                    ps, lhsT=w_hh_sb[j][:, k, gi * P:(gi + 1) * P],
                                    rhs=h_cur[k],
                                    start=False, stop=(k == KH - 1))
                        gt = gp.tile([P, Bc], dt.float32, tag=f"g{gi}")
                        nc.scalar.activation(gt, ps, GATE_FUNCS[gi],
                                             bias=bias_sb[:, m:m + 1])
                        acts.append(gt)
                    i_t, f_t, g_t, o_t = acts
                    cn = sp.tile([P, Bc], dt.float32, tag=f"c{j}")
                    if t == 0:
                        nc.vector.tensor_mul(cn, i_t, g_t)
                    else:
                        u = tp.tile([P, Bc], dt.float32, tag="u")
                        nc.vector.tensor_mul(u, i_t, g_t)
                        v = tp.tile([P, Bc], dt.float32, tag="v")
                        nc.vector.tensor_mul(v, f_t, c_cur[j])
                        nc.vector.tensor_add(cn, u, v)
                    th = tp.tile([P, Bc], dt.float32, tag="th")
                    nc.scalar.activation(th, cn, AF.Tanh)
                    hn = sp.tile([P, Bc], io_dt, tag=f"h{j}")
                    nc.vector.tensor_mul(hn, o_t, th)
                    nc.sync.dma_start(out=yT[t, j * P:(j + 1) * P, :], in_=hn)
                    h_next.append(hn)
                    c_next.append(cn)
                h_cur, c_cur = h_next, c_next

    nc.compile()
    return nc


def _get_nc(T=FRAME_LENGTH, Bc=BC, mode=MM_MODE):
    key = (T, Bc, mode)
    if key not in _CACHE:
        _CACHE[key] = _build(T, Bc, mode)
    return _CACHE[key]


def _prep_inputs(embed_feats, w_ih_l, w_hh_l, b_ih_l, b_hh_l,
                 w_ih_r, w_hh_r, b_ih_r, b_hh_r, mode):
    import ml_dtypes

    io_np = ml_dtypes.bfloat16 if mode == "bf16" else np.float32
    T = embed_feats.shape[1]

    w = {
        0: (np.asarray(w_ih_l), np.asarray(w_hh_l),
            np.asarray(b_ih_l) + np.asarray(b_hh_l)),
        1: (np.asarray(w_ih_r), np.asarray(w_hh_r),
            np.asarray(b_ih_r) + np.asarray(b_hh_r)),
    }
    x = np.asarray(embed_feats)

    # j-major column permutation of the 4H gate dim: block j holds the four
    # gates' columns for hidden chunk j, so each j-chunk loads contiguously
    j_idx, g_idx, c_idx = np.meshgrid(
        np.arange(NJ), np.arange(4), np.arange(P), indexing="ij")
    perm = (g_idx * (NJ * P) + j_idx * P + c_idx).reshape(-1)

    in_maps = []
    for c in range(NCORES):
        d, s = c // NSHARD, c % NSHARD
        xs = x[s * BC:(s + 1) * BC]
        if d == 1:
            xs = xs[:, ::-1]
        xT = np.ascontiguousarray(xs.transpose(1, 2, 0)).astype(io_np)
        w_ihT = np.ascontiguousarray(w[d][0].T[:, perm]).astype(io_np)
        w_hhT = np.ascontiguousarray(w[d][1].T[:, perm]).astype(io_np)
        bias = np.ascontiguousarray(
            w[d][2].astype(np.float32).reshape(NM, P).T)
        in_maps.append({"xT": xT, "w_ih": w_ihT, "w_hh": w_hhT, "bias": bias})
    return in_maps, T


def _run(inputs, mode=MM_MODE, trace=False, trace_kwargs=None):
    from concourse.bass_utils import run_bass_kernel_spmd

    in_maps, T = _prep_inputs(mode=mode, **inputs)
    nc = _get_nc(T=T, mode=mode)
    res = run_bass_kernel_spmd(nc, in_maps, list(range(NCORES)),
                               trace=trace, **(trace_kwargs or {}))

    out = np.empty((BATCH, T, 2 * HIDDEN), np.float32)
    for c in range(NCORES):
        d, s = c // NSHARD, c % NSHARD
        yt = np.asarray(res.results[c]["yT"], dtype=np.float32)  # [T, H, Bc]
        arr = yt.transpose(2, 0, 1)                              # [Bc, T, H]
        if d == 1:
            arr = arr[:, ::-1]
        out[s * BC:(s + 1) * BC, :, d * HIDDEN:(d + 1) * HIDDEN] = arr
    return out, res


def kernel(**inputs):
    out, _ = _run(inputs)
    return out



# revision 5
# speedup vs baseline: 1.1238x; 1.1238x over previous
"""Bidirectional LSTM encoder (nn_BiEncode) as a Bass/Tile kernel on 8 trn2 cores.

Sharding: direction-split x batch-split. Cores 0-3 run the LEFT (forward-time)
direction on batch shards 0-3 (512 rows each); cores 4-7 run the RIGHT
direction (time-reversed input, handled host-side) on the same batch shards.
Every core runs the identical SPMD program; direction differences live
entirely in the data it is fed (weights + time-reversed x).

Device layout: everything is kept "transposed" (feature dim on partitions,
batch on the free dim) so the scan needs no on-chip transposes:
  x fed as xT[t, i, b], weights as W^T, h/c as [H, B] tiles, output written
  as yT[t, h, b] and un-transposed on the host.

Per timestep the full gate pre-activation g^T[4H, B] is computed as 12
PSUM-accumulated f32r matmuls per 128-row gate tile (8 k-tiles of
x-projection + 4 k-tiles of the recurrent term) -- the input projection is
fused into the scan, so no pre-activation tensor is ever materialized. ACT
applies sigmoid/tanh straight out of PSUM; DVE does the cell update.

Startup-bandwidth optimization: the kernel is PE-bound at a flat 227ns
matmul cadence; the only slack is the DMA-bound startup ramp (~38us with
f32 weights: 16MB of critical bytes at ~400GB/s/core, all 8 cores hammering
HBM at once). Weights are therefore shipped as fp16 (half the bytes,
~6e-5 max relative rounding - 4x better than bf16) and upcast on-chip to
f32r by the otherwise-idle DVE/ACT engines before the PE needs them.
y is DMA'd out as fp16 (halves the output stream + drain tail).
"""

import os

import numpy as np

FRAME_LENGTH = 26
HIDDEN = 512
INPUT = 1024
BATCH = 2048

NCORES = 8
NSHARD = 4                 # batch shards per direction group
BC = BATCH // NSHARD       # 512 batch rows per core

P = 128
KI = INPUT // P            # 8  k-tiles for the input projection
KH = HIDDEN // P           # 4  k-tiles for the recurrent matmul
NJ = HIDDEN // P           # 4  hidden chunks
NM = 4 * HIDDEN // P       # 16 gate m-tiles

# "f32r": fp32 storage, PE in float32r (full-rate at N>=256, ~tf32 precision)
# "bf16": bf16 storage+PE (half DMA/SBUF), fp32 PSUM accumulation
MM_MODE = os.environ.get("BASS_LSTM_MM", "f32r")
W16 = os.environ.get("BASS_LSTM_W16", "1") == "1"   # ship weights fp16
Y16 = os.environ.get("BASS_LSTM_Y16", "1") == "1"   # output y as fp16

_CACHE = {}


def _build(T, Bc, mode, w16, y16):
    import concourse.mybir as mybir
    import concourse.tile as tile
    from concourse import bacc

    dt = mybir.dt
    AF = mybir.ActivationFunctionType

    # matmul-operand storage dtype; the BIR verifier requires fp32r matmul
    # inputs to be produced as fp32r, so x/w/h carry it end-to-end
    io_dt = dt.bfloat16 if mode == "bf16" else dt.float32r
    w16 = w16 and mode == "f32r"
    w_dt = dt.float16 if w16 else io_dt
    y_dt = dt.float16 if y16 else io_dt

    nc = bacc.Bacc("TRN2", target_bir_lowering=False, debug=False,
                   num_devices=NCORES)
    xT = nc.dram_tensor("xT", [T, INPUT, Bc], io_dt, kind="ExternalInput").ap()
    w_ih = nc.dram_tensor("w_ih", [INPUT, 4 * HIDDEN], w_dt,
                          kind="ExternalInput").ap()
    w_hh = nc.dram_tensor("w_hh", [HIDDEN, 4 * HIDDEN], w_dt,
                          kind="ExternalInput").ap()
    bias = nc.dram_tensor("bias", [P, NM], dt.float32, kind="ExternalInput").ap()
    yT = nc.dram_tensor("yT", [T, HIDDEN, Bc], y_dt, kind="ExternalOutput").ap()

    with tile.TileContext(nc) as tc:
        with tc.tile_pool(name="wpool", bufs=1) as wp, \
             tc.tile_pool(name="wstage", bufs=2) as ws, \
             tc.tile_pool(name="xpool", bufs=2) as xp, \
             tc.tile_pool(name="cstate", bufs=1) as cp, \
             tc.tile_pool(name="hstate", bufs=2) as hp, \
             tc.tile_pool(name="gates", bufs=2) as gp, \
             tc.tile_pool(name="tmp", bufs=1) as tp, \
             tc.tile_pool(name="psum", bufs=2, space="PSUM") as pp:

            # Startup-latency ordering: the j=0 weight chunk and t=0 x slices
            # are DMA'd first so the PE can start as early as possible.
            # Host pre-permutes W columns j-major (see _prep_inputs) so each
            # j-chunk is one contiguous DMA.
            w_ih_r = w_ih.rearrange("(k p) n -> p k n", p=P)
            w_hh_r = w_hh.rearrange("(k p) n -> p k n", p=P)

            bias_sb = wp.tile([P, NM], dt.float32, tag="bias")
            nc.sync.dma_start(out=bias_sb, in_=bias[:, :])

            def upcast(dst_f32r, src_f16):
                # fp16 -> f32r convert (f32r-typed output keeps the BIR
                # verifier happy about fp32r matmul operands)
                nc.vector.tensor_copy(dst_f32r, src_f16)

            # t=0 x rides the regular x-pipeline tile, filled by per-k slice
            # DMAs so the slices stream while weights convert.
            w_ih_sb = []          # [j] -> [P, KI, 4P] f32r tile
            w_hh_sb = []          # [j] -> [P, KH, 4P] f32r tile
            if w16:
                wf0 = ws.tile([P, KI, 4 * P], dt.float16, tag="ws")
                nc.sync.dma_start(out=wf0, in_=w_ih_r[:, :, 0:4 * P])
                wt0 = wp.tile([P, KI, 4 * P], io_dt, tag="wih0")
                upcast(wt0, wf0)
                w_ih_sb.append(wt0)
            xt0 = xp.tile([P, KI, Bc], io_dt, tag="x")
            for k in range(KI):
                nc.sync.dma_start(
                    out=xt0[:, k, :], in_=xT[0, k * P:(k + 1) * P, :])
            if not w16:
                wt0 = wp.tile([P, KI, 4 * P], io_dt, tag="wih0")
                nc.sync.dma_start(out=wt0, in_=w_ih_r[:, :, 0:4 * P])
                w_ih_sb.append(wt0)
            # prefetch t=1's x before the remaining weight triggers queue up
            xt1 = None
            if T > 1:
                xt1 = xp.tile([P, KI, Bc], io_dt, tag="x")
                nc.sync.dma_start(
                    out=xt1, in_=xT[1].rearrange("(k p) b -> p k b", p=P))
            # remaining w_ih chunks (sync queue), then w_hh (scalar engine's
            # HWDGE queue, in parallel with the sync-engine trigger stream).
            # All upcasts run on DVE in need-order: wih j=0..3, whh j=0..3.
            for j in range(1, NJ):
                wt = wp.tile([P, KI, 4 * P], io_dt, tag=f"wih{j}")
                if w16:
                    wf = ws.tile([P, KI, 4 * P], dt.float16, tag="ws")
                    nc.sync.dma_start(
                        out=wf, in_=w_ih_r[:, :, j * 4 * P:(j + 1) * 4 * P])
                    upcast(wt, wf)
                else:
                    nc.sync.dma_start(
                        out=wt, in_=w_ih_r[:, :, j * 4 * P:(j + 1) * 4 * P])
                w_ih_sb.append(wt)
            for j in range(NJ):
                wt = wp.tile([P, KH, 4 * P], io_dt, tag=f"whh{j}")
                if w16:
                    wf = ws.tile([P, KH, 4 * P], dt.float16, tag="whs")
                    nc.scalar.dma_start(
                        out=wf, in_=w_hh_r[:, :, j * 4 * P:(j + 1) * 4 * P])
                    upcast(wt, wf)
                else:
                    nc.scalar.dma_start(
                        out=wt, in_=w_hh_r[:, :, j * 4 * P:(j + 1) * 4 * P])
                w_hh_sb.append(wt)

            # h0 = c0 = 0, so step 0 skips the recurrent matmuls and the
            # f*c term entirely -- no initial state tiles needed (memset
            # can't produce float32r anyway).
            h_cur, c_cur = [], []

            GATE_FUNCS = (AF.Sigmoid, AF.Sigmoid, AF.Tanh, AF.Sigmoid)

            for t in range(T):
                if t == 0:
                    xt = xt0
                elif t == 1:
                    xt = xt1
                else:
                    xt = xp.tile([P, KI, Bc], io_dt, tag="x")
                    nc.sync.dma_start(
                        out=xt, in_=xT[t].rearrange("(k p) b -> p k b", p=P))

                h_next, c_next = [], []
                for j in range(NJ):
                    acts = []
                    for gi in range(4):
                        m = gi * NJ + j
                        ps = pp.tile([P, Bc], dt.float32, tag=f"ps{gi}")
                        for k in range(KI):
                            nc.tensor.matmul(
                                ps, lhsT=w_ih_sb[j][:, k, gi * P:(gi + 1) * P],
                                rhs=xt[:, k, :],
                                start=(k == 0),
                                stop=(t == 0 and k == KI - 1))
                        if t > 0:
                            for k in range(KH):
                                nc.tensor.matmul(
                                    ps, lhsT=w_hh_sb[j][:, k, gi * P:(gi + 1) * P],
                                    rhs=h_cur[k],
                                    start=False, stop=(k == KH - 1))
                        gt = gp.tile([P, Bc], dt.float32, tag=f"g{gi}")
                        nc.scalar.activation(gt, ps, GATE_FUNCS[gi],
                                             bias=bias_sb[:, m:m + 1])
                        acts.append(gt)
                    i_t, f_t, g_t, o_t = acts
                    cn = cp.tile([P, Bc], dt.float32, tag=f"c{j}")
                    if t == 0:
                        nc.vector.tensor_mul(cn, i_t, g_t)
                    else:
                        u = tp.tile([P, Bc], dt.float32, tag="u")
                        nc.vector.tensor_mul(u, i_t, g_t)
                        v = tp.tile([P, Bc], dt.float32, tag="v")
                        nc.vector.tensor_mul(v, f_t, c_cur[j])
                        nc.vector.tensor_add(cn, u, v)
                    th = tp.tile([P, Bc], dt.float32, tag="th")
                    nc.scalar.activation(th, cn, AF.Tanh)
                    hn = hp.tile([P, Bc], io_dt, tag=f"h{j}")
                    nc.vector.tensor_mul(hn, o_t, th)
                    if y16:
                        yh = tp.tile([P, Bc], dt.float16, tag="yh")
                        nc.vector.tensor_copy(yh, hn)
                        nc.sync.dma_start(out=yT[t, j * P:(j + 1) * P, :],
                                          in_=yh)
                    else:
                        nc.sync.dma_start(out=yT[t, j * P:(j + 1) * P, :],
                                          in_=hn)
                    h_next.append(hn)
                    c_next.append(cn)
                h_cur, c_cur = h_next, c_next

    nc.compile()
    return nc


def _get_nc(T=FRAME_LENGTH, Bc=BC, mode=MM_MODE, w16=W16, y16=Y16):
    key = (T, Bc, mode, w16, y16)
    if key not in _CACHE:
        _CACHE[key] = _build(T, Bc, mode, w16, y16)
    return _CACHE[key]


def _prep_inputs(embed_feats, w_ih_l, w_hh_l, b_ih_l, b_hh_l,
                 w_ih_r, w_hh_r, b_ih_r, b_hh_r, mode, w16):
    import ml_dtypes

    io_np = ml_dtypes.bfloat16 if mode == "bf16" else np.float32
    w_np = np.float16 if (w16 and mode == "f32r") else io_np
    T = embed_feats.shape[1]

    w = {
        0: (np.asarray(w_ih_l), np.asarray(w_hh_l),
            np.asarray(b_ih_l) + np.asarray(b_hh_l)),
        1: (np.asarray(w_ih_r), np.asarray(w_hh_r),
            np.asarray(b_ih_r) + np.asarray(b_hh_r)),
    }
    x = np.asarray(embed_feats)

    # j-major column permutation of the 4H gate dim: block j holds the four
    # gates' columns for hidden chunk j, so each j-chunk loads contiguously
    j_idx, g_idx, c_idx = np.meshgrid(
        np.arange(NJ), np.arange(4), np.arange(P), indexing="ij")
    perm = (g_idx * (NJ * P) + j_idx * P + c_idx).reshape(-1)

    in_maps = []
    for c in range(NCORES):
        d, s = c // NSHARD, c % NSHARD
        xs = x[s * BC:(s + 1) * BC]
        if d == 1:
            xs = xs[:, ::-1]
        xT = np.ascontiguousarray(xs.transpose(1, 2, 0)).astype(io_np)
        w_ihT = np.ascontiguousarray(w[d][0].T[:, perm]).astype(w_np)
        w_hhT = np.ascontiguousarray(w[d][1].T[:, perm]).astype(w_np)
        bias = np.ascontiguousarray(
            w[d][2].astype(np.float32).reshape(NM, P).T)
        in_maps.append({"xT": xT, "w_ih": w_ihT, "w_hh": w_hhT, "bias": bias})
    return in_maps, T


def _run(inputs, mode=MM_MODE, trace=False, trace_kwargs=None):
    from concourse.bass_utils import run_bass_kernel_spmd

    in_maps, T = _prep_inputs(mode=mode, w16=W16, **inputs)
    nc = _get_nc(T=T, mode=mode)
    res = run_bass_kernel_spmd(nc, in_maps, list(range(NCORES)),
                               trace=trace, **(trace_kwargs or {}))

    out = np.empty((BATCH, T, 2 * HIDDEN), np.float32)
    for c in range(NCORES):
        d, s = c // NSHARD, c % NSHARD
        yt = np.asarray(res.results[c]["yT"], dtype=np.float32)  # [T, H, Bc]
        arr = yt.transpose(2, 0, 1)                              # [Bc, T, H]
        if d == 1:
            arr = arr[:, ::-1]
        out[s * BC:(s + 1) * BC, :, d * HIDDEN:(d + 1) * HIDDEN] = arr
    return out, res


def kernel(**inputs):
    out, _ = _run(inputs)
    return out


# revision 9
# speedup vs baseline: 1.1325x; 1.0078x over previous
"""Bidirectional LSTM encoder (nn_BiEncode) as a Bass/Tile kernel on 8 trn2 cores.

Sharding: direction-split x batch-split. Cores 0-3 run the LEFT (forward-time)
direction on batch shards 0-3 (512 rows each); cores 4-7 run the RIGHT
direction (time-reversed input, handled host-side) on the same batch shards.
Every core runs the identical SPMD program; direction differences live
entirely in the data it is fed (weights + time-reversed x).

Device layout: everything is kept "transposed" (feature dim on partitions,
batch on the free dim) so the scan needs no on-chip transposes:
  x fed as xT[t, i, b], weights as W^T, h/c as [H, B] tiles, output written
  as yT[t, h, b] and un-transposed on the host.

Per timestep the full gate pre-activation g^T[4H, B] is computed as 12
PSUM-accumulated f32r matmuls per 128-row gate tile (8 k-tiles of
x-projection + 4 k-tiles of the recurrent term) -- the input projection is
fused into the scan, so no pre-activation tensor is ever materialized. ACT
applies sigmoid/tanh straight out of PSUM; DVE does the cell update.

Startup-bandwidth optimization: the kernel is PE-bound at a flat 227ns
matmul cadence; the only slack is the DMA-bound startup ramp (~38us with
f32 weights: 16MB of critical bytes at ~400GB/s/core, all 8 cores hammering
HBM at once). Weights are therefore shipped as fp16 (half the bytes,
~6e-5 max relative rounding - 4x better than bf16) and upcast on-chip to
f32r by the otherwise-idle DVE/ACT engines before the PE needs them.
y is DMA'd out as fp16 (halves the output stream + drain tail).
"""

import os

import numpy as np

FRAME_LENGTH = 26
HIDDEN = 512
INPUT = 1024
BATCH = 2048

NCORES = 8
NSHARD = 4                 # batch shards per direction group
BC = BATCH // NSHARD       # 512 batch rows per core

P = 128
KI = INPUT // P            # 8  k-tiles for the input projection
KH = HIDDEN // P           # 4  k-tiles for the recurrent matmul
NJ = HIDDEN // P           # 4  hidden chunks
NM = 4 * HIDDEN // P       # 16 gate m-tiles

# "f32r": fp32 storage, PE in float32r (full-rate at N>=256, ~tf32 precision)
# "bf16": bf16 storage+PE (half DMA/SBUF), fp32 PSUM accumulation
MM_MODE = os.environ.get("BASS_LSTM_MM", "f32r")
W16 = os.environ.get("BASS_LSTM_W16", "1") == "1"   # ship weights fp16
Y16 = os.environ.get("BASS_LSTM_Y16", "1") == "1"   # output y as fp16

_CACHE = {}


def _build(T, Bc, mode, w16, y16):
    import concourse.mybir as mybir
    import concourse.tile as tile
    from concourse import bacc

    dt = mybir.dt
    AF = mybir.ActivationFunctionType

    # matmul-operand storage dtype; the BIR verifier requires fp32r matmul
    # inputs to be produced as fp32r, so x/w/h carry it end-to-end
    io_dt = dt.bfloat16 if mode == "bf16" else dt.float32r
    w16 = w16 and mode == "f32r"
    w_dt = dt.float16 if w16 else io_dt
    y_dt = dt.float16 if y16 else io_dt

    nc = bacc.Bacc("TRN2", target_bir_lowering=False, debug=False,
                   num_devices=NCORES)
    xT = nc.dram_tensor("xT", [T, INPUT, Bc], io_dt, kind="ExternalInput").ap()
    w_ih = nc.dram_tensor("w_ih", [INPUT, 4 * HIDDEN], w_dt,
                          kind="ExternalInput").ap()
    w_hh = nc.dram_tensor("w_hh", [HIDDEN, 4 * HIDDEN], w_dt,
                          kind="ExternalInput").ap()
    bias = nc.dram_tensor("bias", [P, NM], dt.float32, kind="ExternalInput").ap()
    yT = nc.dram_tensor("yT", [T, HIDDEN, Bc], y_dt, kind="ExternalOutput").ap()

    with tile.TileContext(nc) as tc:
        with tc.tile_pool(name="wpool", bufs=1) as wp, \
             tc.tile_pool(name="xpool", bufs=2) as xp, \
             tc.tile_pool(name="work", bufs=1) as wk, \
             tc.tile_pool(name="psum", bufs=2, space="PSUM") as pp:

            # Startup-latency plan (the kernel is PE-bound; all slack is in
            # the DMA-bound ramp): DMA trigger issue costs ~0.64us each on an
            # engine sequencer, so the startup transfers are spread across
            # four engines' HWDGE queues to issue in parallel:
            #   sync:   w_ih fp16 (j0 per-k slices first, then j1..j3)
            #   scalar: w_hh fp16
            #   gpsimd: t=0 x slices + t=1 x prefetch
            # (DVE does the fp16->f32r upcasts but cannot issue DMAs)
            # Subtile dependency tracking lets the first matmul start once
            # x[k=0] and the k=0 slice of the j0 weight upcast land.
            # Host pre-permutes W columns j-major (see _prep_inputs) so each
            # j-chunk is one contiguous DMA.
            w_ih_r = w_ih.rearrange("(k p) n -> p k n", p=P)
            w_hh_r = w_hh.rearrange("(k p) n -> p k n", p=P)

            bias_sb = wp.tile([P, NM], dt.float32, tag="bias")
            nc.sync.dma_start(out=bias_sb, in_=bias[:, :])

            def upcast(dst_f32r, src_f16):
                # fp16 -> f32r convert (f32r-typed output keeps the BIR
                # verifier happy about fp32r matmul operands)
                nc.vector.tensor_copy(dst_f32r, src_f16)

            xt0 = xp.tile([P, KI, Bc], io_dt, tag="x")
            for k in range(KI):
                nc.gpsimd.dma_start(
                    out=xt0[:, k, :], in_=xT[0, k * P:(k + 1) * P, :])

            w_ih_sb = []          # [j] -> [P, KI, 4P] f32r tile
            w_hh_sb = []          # [j] -> [P, KH, 4P] f32r tile
            if w16:
                # j=0 flows per-k for minimum first-matmul latency
                wt0 = wp.tile([P, KI, 4 * P], io_dt, tag="wih0")
                for k in range(KI):
                    wfk = wp.tile([P, 4 * P], dt.float16, tag="wsk", bufs=4)
                    nc.sync.dma_start(out=wfk, in_=w_ih_r[:, k, 0:4 * P])
                    upcast(wt0[:, k, :], wfk)
                w_ih_sb.append(wt0)
                for j in range(1, NJ):
                    wf = wp.tile([P, KI, 4 * P], dt.float16, tag="ws", bufs=2)
                    nc.sync.dma_start(
                        out=wf, in_=w_ih_r[:, :, j * 4 * P:(j + 1) * 4 * P])
                    wt = wp.tile([P, KI, 4 * P], io_dt, tag=f"wih{j}")
                    upcast(wt, wf)
                    w_ih_sb.append(wt)
                for j in range(NJ):
                    wf = wp.tile([P, KH, 4 * P], dt.float16, tag="whs", bufs=2)
                    nc.scalar.dma_start(
                        out=wf, in_=w_hh_r[:, :, j * 4 * P:(j + 1) * 4 * P])
                    wt = wp.tile([P, KH, 4 * P], io_dt, tag=f"whh{j}")
                    upcast(wt, wf)
                    w_hh_sb.append(wt)
            else:
                for j in range(NJ):
                    wt = wp.tile([P, KI, 4 * P], io_dt, tag=f"wihf{j}")
                    nc.sync.dma_start(
                        out=wt, in_=w_ih_r[:, :, j * 4 * P:(j + 1) * 4 * P])
                    w_ih_sb.append(wt)
                for j in range(NJ):
                    wt = wp.tile([P, KH, 4 * P], io_dt, tag=f"whh{j}")
                    nc.scalar.dma_start(
                        out=wt, in_=w_hh_r[:, :, j * 4 * P:(j + 1) * 4 * P])
                    w_hh_sb.append(wt)
            # prefetch t=1's x on the gpsimd engine's queue
            xt1 = None
            if T > 1:
                xt1 = xp.tile([P, KI, Bc], io_dt, tag="x")
                nc.gpsimd.dma_start(
                    out=xt1, in_=xT[1].rearrange("(k p) b -> p k b", p=P))

            # h0 = c0 = 0, so step 0 skips the recurrent matmuls and the
            # f*c term entirely -- no initial state tiles needed (memset
            # can't produce float32r anyway).
            h_cur, c_cur = [], []

            GATE_FUNCS = (AF.Sigmoid, AF.Sigmoid, AF.Tanh, AF.Sigmoid)

            for t in range(T):
                if t == 0:
                    xt = xt0
                elif t == 1:
                    xt = xt1
                else:
                    xt = xp.tile([P, KI, Bc], io_dt, tag="x")
                    nc.sync.dma_start(
                        out=xt, in_=xT[t].rearrange("(k p) b -> p k b", p=P))

                h_next, c_next = [], []
                for j in range(NJ):
                    acts = []
                    for gi in range(4):
                        m = gi * NJ + j
                        ps = pp.tile([P, Bc], dt.float32, tag=f"ps{gi}")
                        for k in range(KI):
                            nc.tensor.matmul(
                                ps, lhsT=w_ih_sb[j][:, k, gi * P:(gi + 1) * P],
                                rhs=xt[:, k, :],
                                start=(k == 0),
                                stop=(t == 0 and k == KI - 1))
                        if t > 0:
                            for k in range(KH):
                                nc.tensor.matmul(
                                    ps, lhsT=w_hh_sb[j][:, k, gi * P:(gi + 1) * P],
                                    rhs=h_cur[k],
                                    start=False, stop=(k == KH - 1))
                        gt = wk.tile([P, Bc], dt.float32, tag=f"g{gi}",
                                     bufs=2)
                        nc.scalar.activation(gt, ps, GATE_FUNCS[gi],
                                             bias=bias_sb[:, m:m + 1])
                        acts.append(gt)
                    i_t, f_t, g_t, o_t = acts
                    cn = wk.tile([P, Bc], dt.float32, tag=f"c{j}")
                    if t == 0:
                        nc.vector.tensor_mul(cn, i_t, g_t)
                    else:
                        u = wk.tile([P, Bc], dt.float32, tag="u")
                        nc.vector.tensor_mul(u, i_t, g_t)
                        v = wk.tile([P, Bc], dt.float32, tag="v")
                        nc.vector.tensor_mul(v, f_t, c_cur[j])
                        nc.vector.tensor_add(cn, u, v)
                    th = wk.tile([P, Bc], dt.float32, tag="th")
                    nc.scalar.activation(th, cn, AF.Tanh)
                    hn = wk.tile([P, Bc], io_dt, tag=f"h{j}", bufs=2)
                    nc.vector.tensor_mul(hn, o_t, th)
                    if y16:
                        yh = wk.tile([P, Bc], dt.float16, tag="yh")
                        nc.vector.tensor_copy(yh, hn)
                        nc.sync.dma_start(out=yT[t, j * P:(j + 1) * P, :],
                                          in_=yh)
                    else:
                        nc.sync.dma_start(out=yT[t, j * P:(j + 1) * P, :],
                                          in_=hn)
                    h_next.append(hn)
                    c_next.append(cn)
                h_cur, c_cur = h_next, c_next

    nc.compile()
    return nc


def _get_nc(T=FRAME_LENGTH, Bc=BC, mode=MM_MODE, w16=W16, y16=Y16):
    key = (T, Bc, mode, w16, y16)
    if key not in _CACHE:
        _CACHE[key] = _build(T, Bc, mode, w16, y16)
    return _CACHE[key]


def _prep_inputs(embed_feats, w_ih_l, w_hh_l, b_ih_l, b_hh_l,
                 w_ih_r, w_hh_r, b_ih_r, b_hh_r, mode, w16):
    import ml_dtypes

    io_np = ml_dtypes.bfloat16 if mode == "bf16" else np.float32
    w_np = np.float16 if (w16 and mode == "f32r") else io_np
    T = embed_feats.shape[1]

    w = {
        0: (np.asarray(w_ih_l), np.asarray(w_hh_l),
            np.asarray(b_ih_l) + np.asarray(b_hh_l)),
        1: (np.asarray(w_ih_r), np.asarray(w_hh_r),
            np.asarray(b_ih_r) + np.asarray(b_hh_r)),
    }
    x = np.asarray(embed_feats)

    # j-major column permutation of the 4H gate dim: block j holds the four
    # gates' columns for hidden chunk j, so each j-chunk loads contiguously
    j_idx, g_idx, c_idx = np.meshgrid(
        np.arange(NJ), np.arange(4), np.arange(P), indexing="ij")
    perm = (g_idx * (NJ * P) + j_idx * P + c_idx).reshape(-1)

    in_maps = []
    for c in range(NCORES):
        d, s = c // NSHARD, c % NSHARD
        xs = x[s * BC:(s + 1) * BC]
        if d == 1:
            xs = xs[:, ::-1]
        xT = np.ascontiguousarray(xs.transpose(1, 2, 0)).astype(io_np)
        w_ihT = np.ascontiguousarray(w[d][0].T[:, perm]).astype(w_np)
        w_hhT = np.ascontiguousarray(w[d][1].T[:, perm]).astype(w_np)
        bias = np.ascontiguousarray(
            w[d][2].astype(np.float32).reshape(NM, P).T)
        in_maps.append({"xT": xT, "w_ih": w_ihT, "w_hh": w_hhT, "bias": bias})
    return in_maps, T


def _run(inputs, mode=MM_MODE, trace=False, trace_kwargs=None):
    from concourse.bass_utils import run_bass_kernel_spmd

    in_maps, T = _prep_inputs(mode=mode, w16=W16, **inputs)
    nc = _get_nc(T=T, mode=mode)
    res = run_bass_kernel_spmd(nc, in_maps, list(range(NCORES)),
                               trace=trace, **(trace_kwargs or {}))

    out = np.empty((BATCH, T, 2 * HIDDEN), np.float32)
    for c in range(NCORES):
        d, s = c // NSHARD, c % NSHARD
        yt = np.asarray(res.results[c]["yT"], dtype=np.float32)  # [T, H, Bc]
        arr = yt.transpose(2, 0, 1)                              # [Bc, T, H]
        if d == 1:
            arr = arr[:, ::-1]
        out[s * BC:(s + 1) * BC, :, d * HIDDEN:(d + 1) * HIDDEN] = arr
    return out, res


def kernel(**inputs):
    out, _ = _run(inputs)
    return out
